# revision 1
# baseline (speedup 1.0000x reference)
"""Trainium2 Bass kernel for a hybrid classical/quantum head.

Math: the reference is  out = Q(tanh(X @ Wpre.T + bpre) * pi/2) @ Wpost.T + bpost
where Q() simulates a 10-qubit circuit: H on all wires, per-sample RY(theta_w),
then 6 layers of (CNOT chain + shared RY(qw)), returning PauliZ expvals.

Restructuring used here:
  * After H + per-sample RY, the state is a PRODUCT state:
      s2[j] = prod_w v_w(bit_w(j)),  v_w(0)=cos(phi_w), v_w(1)=sin(phi_w),
      phi_w = theta_w/2 + pi/4,  theta_w = tanh(pre)*pi/2.
  * Everything after is a fixed linear operator A (1024x1024) that depends only
    on q_params -> built host-side in fp64 (tiny), shipped as fp16.
  * z_w = sum_j sign_w(j) * (A s2)_j^2, and the post-linear folds in:
      out[s, c] = sum_j d[c, j] * y[s, j]^2 + bpost[c],  d = Wpost @ Sgn.

Per-core device pipeline (data-parallel over batch, 1024 samples/core):
  prenet matmul (f32, PE) -> tanh/sin (ACT) -> kron-factor build (GPSIMD)
  -> broadcast-AP PE transposes -> expand to S2^T fp16 (DVE)
  -> Y^T = A @ S2^T (fp16 PE matmul, f32 accum) -> square (ACT)
  -> d-contraction (fp16 PE matmul) -> +bias -> out^T (2, 1024) f32.
"""

import numpy as np

N_QUBITS = 10
Q_DEPTH = 6
MAX_LAYERS = 15
DIM = 2**N_QUBITS
N_CORES = 8
B_FULL = 8192
F_IN = 512
N_CLS = 2
BC = B_FULL // N_CORES  # 1024 samples per core
P = 128

_CACHE = {}


def _build_A(q_params):
    """Fixed circuit operator after the per-sample RY layer, fp64 on host."""
    qp = np.asarray(q_params, np.float64)
    qw = qp.reshape(MAX_LAYERS, N_QUBITS)
    N = N_QUBITS

    def apply_1q(M, U, w):
        a, b = 2**w, 2 ** (N - 1 - w)
        M = M.reshape(a, 2, b, DIM)
        M = np.einsum('ij,ajbk->aibk', U, M)
        return M.reshape(DIM, DIM)

    def apply_cnot(M, c, t):
        M = M.reshape(2**c, 2, 2 ** (t - c - 1), 2, 2 ** (N - 1 - t), DIM)
        M = np.stack([M[:, 0], np.flip(M[:, 1], axis=2)], axis=1)
        return M.reshape(DIM, DIM)

    def ry(th):
        c, s = np.cos(th / 2), np.sin(th / 2)
        return np.array([[c, -s], [s, c]])

    A = np.eye(DIM)
    for k in range(Q_DEPTH):
        for i in range(0, N - 1, 2):
            A = apply_cnot(A, i, i + 1)
        for i in range(1, N - 1, 2):
            A = apply_cnot(A, i, i + 1)
        for w in range(N):
            A = apply_1q(A, ry(qw[k + 1, w]), w)
    return A


def _build_bass():
    import concourse.mybir as mybir
    from concourse import bacc
    from concourse.tile import TileContext

    dt = mybir.dt
    AF = mybir.ActivationFunctionType
    ALU = mybir.AluOpType
    PI = float(np.pi)

    nc = bacc.Bacc()
    xT = nc.dram_tensor("xT", [F_IN, BC], dt.float32, kind="ExternalInput")
    wpre = nc.dram_tensor("wpre", [4, P, N_QUBITS], dt.float32, kind="ExternalInput")
    bpre = nc.dram_tensor("bpre", [P, N_QUBITS], dt.float32, kind="ExternalInput")
    aT = nc.dram_tensor("aT", [DIM, DIM], dt.float16, kind="ExternalInput")
    dT = nc.dram_tensor("dT", [DIM, N_CLS], dt.float16, kind="ExternalInput")
    bpost = nc.dram_tensor("bpost", [N_CLS, 1], dt.float32, kind="ExternalInput")
    outT = nc.dram_tensor("outT", [N_CLS, BC], dt.float32, kind="ExternalOutput")

    NSUB = BC // P           # 8 sample sub-tiles
    NKT = DIM // P           # 8 k (amplitude) tiles
    NCH = 2                  # two 512-wide sample chunks for the big matmuls
    CW = BC // NCH           # 512

    with TileContext(nc) as tc:
        with (
            tc.tile_pool(name="const", bufs=1) as cpool,
            tc.tile_pool(name="small", bufs=3) as spool,
            tc.tile_pool(name="ps_pre", bufs=2, space="PSUM") as ps_pre,
            tc.tile_pool(name="ps_y", bufs=4, space="PSUM") as ps_y,
            tc.tile_pool(name="ps_o", bufs=1, space="PSUM") as ps_o,
        ):
            bias14 = cpool.tile([P, 1], dt.float32)
            nc.gpsimd.memset(bias14, PI / 4.0)
            bias34 = cpool.tile([P, 1], dt.float32)
            nc.gpsimd.memset(bias34, 3.0 * PI / 4.0)

            wpre_sb = cpool.tile([P, 4, N_QUBITS], dt.float32)
            nc.sync.dma_start(wpre_sb, wpre[:].rearrange("a p q -> p a q"))
            bpre_sb = cpool.tile([P, N_QUBITS], dt.float32)
            nc.sync.dma_start(bpre_sb, bpre[:])
            dT_sb = cpool.tile([P, NKT, N_CLS], dt.float16)
            nc.sync.dma_start(dT_sb, dT[:].rearrange("(t p) c -> p t c", p=P))
            bpost_sb = cpool.tile([N_CLS, 1], dt.float32)
            nc.sync.dma_start(bpost_sb, bpost[:])
            xT_sb = cpool.tile([P, 4, BC], dt.float32)
            nc.sync.dma_start(xT_sb, xT[:].rearrange("(a p) s -> p a s", p=P))
            aT_sb = cpool.tile([P, NKT, DIM], dt.float16)
            nc.sync.dma_start(aT_sb, aT[:].rearrange("(t p) j -> p t j", p=P))

            # persistent staging for prep phase
            q_all = cpool.tile([P, NSUB, N_QUBITS], dt.float32)
            v0_all = cpool.tile([P, NSUB, N_QUBITS], dt.float32)
            v1_all = cpool.tile([P, NSUB, N_QUBITS], dt.float32)
            shi_all = cpool.tile([P, NSUB, 32], dt.float32)
            slo_all = cpool.tile([P, NSUB, 32], dt.float32)
            s2T = [cpool.tile([P, NKT, CW], dt.float16, name=f"s2T{c}") for c in range(NCH)]
            p_all = [cpool.tile([P, NKT, CW], dt.float16, name=f"p_all{c}") for c in range(NCH)]
            outT_sb = cpool.tile([N_CLS, BC], dt.float32)

            # ---- prep: per-sub pipeline ----
            def build_half(dst, sub, wires, eng):
                # dst[:, sub, :]: 32 columns = product over 5 wires, first wire
                # in `wires` ends up the most-significant bit.
                v0 = v0_all[:, sub, :]
                v1 = v1_all[:, sub, :]
                t2 = spool.tile([P, 2], dt.float32, name=f"t2_{sub}", tag=f"t2{eng}")
                t4 = spool.tile([P, 4], dt.float32, name=f"t4_{sub}", tag=f"t4{eng}")
                t8 = spool.tile([P, 8], dt.float32, name=f"t8_{sub}", tag=f"t8{eng}")
                t16 = spool.tile([P, 16], dt.float32, name=f"t16_{sub}", tag=f"t16{eng}")
                w4, w3, w2, w1, w0 = wires[4], wires[3], wires[2], wires[1], wires[0]
                if eng == "v":
                    cp = nc.vector.tensor_copy
                    mul = nc.vector.tensor_scalar_mul
                else:
                    cp = nc.scalar.copy
                    # ACT: out = Copy(in * scale) with per-partition scale AP
                    mul = nc.scalar.mul
                cp(t2[:, 0:1], v0[:, w4:w4 + 1])
                cp(t2[:, 1:2], v1[:, w4:w4 + 1])
                mul(t4[:, 0:2], t2, v0[:, w3:w3 + 1])
                mul(t4[:, 2:4], t2, v1[:, w3:w3 + 1])
                mul(t8[:, 0:4], t4, v0[:, w2:w2 + 1])
                mul(t8[:, 4:8], t4, v1[:, w2:w2 + 1])
                mul(t16[:, 0:8], t8, v0[:, w1:w1 + 1])
                mul(t16[:, 8:16], t8, v1[:, w1:w1 + 1])
                mul(dst[:, sub, 0:16], t16, v0[:, w0:w0 + 1])
                mul(dst[:, sub, 16:32], t16, v1[:, w0:w0 + 1])

            for sub in range(NSUB):
                ch, csub = sub // 4, sub % 4
                csl = slice(csub * P, (csub + 1) * P)
                ssl = slice(sub * P, (sub + 1) * P)
                pre_ps = ps_pre.tile([P, N_QUBITS], dt.float32)
                for ft in range(4):
                    nc.tensor.matmul(
                        pre_ps, xT_sb[:, ft, ssl], wpre_sb[:, ft, :],
                        start=(ft == 0), stop=(ft == 3),
                    )
                preb = spool.tile([P, N_QUBITS], dt.float32, name=f"preb{sub}")
                # preb = pre + b_pre
                nc.vector.scalar_tensor_tensor(
                    preb, pre_ps, 1.0, bpre_sb, ALU.mult, ALU.add
                )
                nc.scalar.activation(q_all[:, sub, :], preb, AF.Tanh)
                # theta = q * pi/2 ; phi = theta/2 + pi/4 ; v0 = cos phi, v1 = sin phi
                nc.scalar.activation(
                    v0_all[:, sub, :], q_all[:, sub, :], AF.Sin,
                    bias=bias34[:, 0:1], scale=PI / 4.0,
                )
                nc.scalar.activation(
                    v1_all[:, sub, :], q_all[:, sub, :], AF.Sin,
                    bias=bias14[:, 0:1], scale=PI / 4.0,
                )
                build_half(shi_all, sub, [0, 1, 2, 3, 4], "v")
                build_half(slo_all, sub, [5, 6, 7, 8, 9], "s")
                # s2 (normal layout, fp16): out[s, a*32+b] = shi[s,a] * slo[s,b]
                s2n = spool.tile([P, DIM], dt.float16, name=f"s2n{sub}", tag="s2n")
                nc.vector.tensor_mul(
                    s2n.rearrange("p (a b) -> p a b", a=32),
                    shi_all[:, sub, :, None].broadcast_to((P, 32, 32)),
                    slo_all[:, sub, None, :].broadcast_to((P, 32, 32)),
                )
                # transpose to (amplitude, sample) layout via DMA xbar
                for kt in range(NKT):
                    nc.sync.dma_start(
                        s2T[ch][:, kt, csl], s2n[:, kt * P:(kt + 1) * P],
                        transpose=True,
                    )

            # ---- stage 5: main matmul Y^T = A @ S2^T + square + d-contraction ----
            for ch in range(NCH):
                for jt in range(NKT):
                    jsl = slice(jt * P, (jt + 1) * P)
                    y_ps = ps_y.tile([P, CW], dt.float32, name=f"y_ps{ch}_{jt}", tag="y")
                    for kt in range(NKT):
                        nc.tensor.matmul(
                            y_ps, aT_sb[:, kt, jsl], s2T[ch][:, kt, :],
                            start=(kt == 0), stop=(kt == NKT - 1),
                        )
                    nc.scalar.activation(p_all[ch][:, jt, :], y_ps, AF.Square)
                out_ps = ps_o.tile([N_CLS, CW], dt.float32, name=f"out_ps{ch}", tag="o")
                for jt in range(NKT):
                    nc.tensor.matmul(
                        out_ps, dT_sb[:, jt, :], p_all[ch][:, jt, :],
                        start=(jt == 0), stop=(jt == NKT - 1),
                    )
                nc.scalar.activation(
                    outT_sb[:, ch * CW:(ch + 1) * CW], out_ps, AF.Identity,
                    bias=bpost_sb[:, 0:1],
                )

            nc.sync.dma_start(outT[:], outT_sb)

    nc.finalize()
    return nc


def _get_nc():
    if "nc" not in _CACHE:
        _CACHE["nc"] = _build_bass()
    return _CACHE["nc"]


def _prepare_in_maps(input_features, W_pre, b_pre, q_params, W_post, b_post):
    X = np.asarray(input_features, np.float32)
    A = _build_A(q_params)
    AT16 = np.ascontiguousarray(A.T).astype(np.float16)

    j = np.arange(DIM)
    sgn = np.stack(
        [1.0 - 2.0 * ((j >> (N_QUBITS - 1 - w)) & 1) for w in range(N_QUBITS)]
    )  # (10, 1024)
    d = np.asarray(W_post, np.float64) @ sgn  # (2, 1024)
    dT16 = np.ascontiguousarray(d.T).astype(np.float16)

    wpre_pack = np.ascontiguousarray(
        np.asarray(W_pre, np.float32).T.reshape(4, P, N_QUBITS)
    )
    bpre_rep = np.ascontiguousarray(
        np.broadcast_to(np.asarray(b_pre, np.float32), (P, N_QUBITS))
    )
    bpost_col = np.asarray(b_post, np.float32).reshape(N_CLS, 1)

    XT = np.asarray(X, np.float32).T  # (512, 8192)
    in_maps = []
    for c in range(N_CORES):
        in_maps.append({
            "xT": np.ascontiguousarray(XT[:, c * BC:(c + 1) * BC]),
            "wpre": wpre_pack,
            "bpre": bpre_rep,
            "aT": AT16,
            "dT": dT16,
            "bpost": bpost_col,
        })
    return in_maps


def run(inputs, trace=False):
    """Run on 8 cores; returns (output (8192, 2) f32, BassKernelResults)."""
    from concourse.bass_utils import run_bass_kernel_spmd

    nc = _get_nc()
    in_maps = _prepare_in_maps(**inputs)
    res = run_bass_kernel_spmd(
        nc, in_maps, core_ids=list(range(N_CORES)), trace=trace
    )
    out = np.empty((B_FULL, N_CLS), np.float32)
    for c in range(N_CORES):
        out[c * BC:(c + 1) * BC, :] = res.results[c]["outT"].T
    return out, res


def kernel(input_features, W_pre, b_pre, q_params, W_post, b_post):
    out, _ = run(dict(
        input_features=input_features, W_pre=W_pre, b_pre=b_pre,
        q_params=q_params, W_post=W_post, b_post=b_post,
    ))
    return out



# revision 11
# speedup vs baseline: 1.8358x; 1.8358x over previous
"""Trainium2 Bass kernel for a hybrid classical/quantum head.

Math: the reference is  out = Q(tanh(X @ Wpre.T + bpre) * pi/2) @ Wpost.T + bpost
where Q() simulates a 10-qubit circuit: H on all wires, per-sample RY(theta_w),
then 6 layers of (CNOT chain + shared RY(qw)), returning PauliZ expvals.

Restructuring used here:
  * After H + per-sample RY, the state is a PRODUCT state:
      s2[j] = prod_w v_w(bit_w(j)),  v_w(0)=cos(phi_w), v_w(1)=sin(phi_w),
      phi_w = theta_w/2 + pi/4,  theta_w = tanh(pre)*pi/2.
    All v are strictly positive (phi in (0, pi/2)), so the product state can
    be built in AMPLITUDE-MAJOR layout directly via a log-domain matmul:
      S2^T[k, s] = exp( sum_r Bits[k, r] * ln(v_r[s]) )
    with Bits a constant (1024, 20) 0/1 bit-selection matrix.  This kills
    the 64 serialized DMA transposes that dominated the previous version.
  * Everything after the per-sample RY layer is a fixed linear operator A
    (1024x1024) depending only on q_params -> built host-side in fp64,
    shipped as fp16.
  * z_w = sum_j sign_w(j) * (A s2)_j^2, and the post-linear folds in:
      out[s, c] = sum_j d[c, j] * y[s, j]^2 + bpost[c],  d = Wpost @ Sgn.

Per-core device pipeline (data-parallel over batch, 1024 samples/core):
  prenet matmul (Wpre stationary, fp16 PE) -> [10,1024] tanh/sin/ln ACT chain
  -> Bits matmul (PE) -> exp (ACT) -> S2^T fp16
  -> Y^T = A @ S2^T (fp16 PE matmul, f32 accum) -> square (DVE)
  -> d-contraction (fp16 PE matmul) -> +bias (ACT Identity) -> out^T f32.
"""

import numpy as np

N_QUBITS = 10
Q_DEPTH = 6
MAX_LAYERS = 15
DIM = 2**N_QUBITS
N_CORES = 8
B_FULL = 8192
F_IN = 512
N_CLS = 2
BC = B_FULL // N_CORES  # 1024 samples per core
P = 128

_CACHE = {}


def _build_A(q_params):
    """Fixed circuit operator after the per-sample RY layer, fp64 on host."""
    qp = np.asarray(q_params, np.float64)
    qw = qp.reshape(MAX_LAYERS, N_QUBITS)
    N = N_QUBITS

    def apply_1q(M, U, w):
        a, b = 2**w, 2 ** (N - 1 - w)
        M = M.reshape(a, 2, b, DIM)
        M = np.einsum('ij,ajbk->aibk', U, M)
        return M.reshape(DIM, DIM)

    def apply_cnot(M, c, t):
        M = M.reshape(2**c, 2, 2 ** (t - c - 1), 2, 2 ** (N - 1 - t), DIM)
        M = np.stack([M[:, 0], np.flip(M[:, 1], axis=2)], axis=1)
        return M.reshape(DIM, DIM)

    def ry(th):
        c, s = np.cos(th / 2), np.sin(th / 2)
        return np.array([[c, -s], [s, c]])

    A = np.eye(DIM)
    for k in range(Q_DEPTH):
        for i in range(0, N - 1, 2):
            A = apply_cnot(A, i, i + 1)
        for i in range(1, N - 1, 2):
            A = apply_cnot(A, i, i + 1)
        for w in range(N):
            A = apply_1q(A, ry(qw[k + 1, w]), w)
    return A


NKT = DIM // P  # 8 amplitude tiles
NFT = F_IN // P  # 4 feature tiles
NCH = 2  # two 512-wide sample chunks (PSUM bank = 512 f32)
CW = BC // NCH  # 512
NW = 2 * N_QUBITS  # 20 selection rows


def _build_bass():
    import concourse.mybir as mybir
    from concourse import bacc
    from concourse.tile import TileContext

    dt = mybir.dt
    AF = mybir.ActivationFunctionType
    PI = float(np.pi)

    nc = bacc.Bacc()
    x16 = nc.dram_tensor("x16", [P, NFT, BC], dt.float16, kind="ExternalInput")
    # wpre columns duplicated (cos half / sin half) so the whole [20, *]
    # tanh/sin/ln chain stays partition-aligned.
    wpre = nc.dram_tensor("wpre", [P, NFT, NW], dt.float16, kind="ExternalInput")
    bpre = nc.dram_tensor("bpre", [NW, 1], dt.float32, kind="ExternalInput")
    bsin = nc.dram_tensor("bsin", [NW, 1], dt.float32, kind="ExternalInput")
    eps = nc.dram_tensor("eps", [NW, 1], dt.float32, kind="ExternalInput")
    bits = nc.dram_tensor("bits", [NW, DIM], dt.float16, kind="ExternalInput")
    a16 = nc.dram_tensor("a16", [P, NKT, DIM], dt.float16, kind="ExternalInput")
    d16 = nc.dram_tensor("d16", [P, NKT, N_CLS], dt.float16, kind="ExternalInput")
    bpost = nc.dram_tensor("bpost", [N_CLS, 1], dt.float32, kind="ExternalInput")
    outT = nc.dram_tensor("outT", [N_CLS, BC], dt.float32, kind="ExternalOutput")

    with TileContext(nc) as tc:
        with (
            tc.tile_pool(name="const", bufs=1) as cpool,
            tc.tile_pool(name="ps_a", bufs=2, space="PSUM") as ps_a,
            tc.tile_pool(name="ps_y", bufs=4, space="PSUM") as ps_y,
            tc.tile_pool(name="ps_o", bufs=2, space="PSUM") as ps_o,
        ):
            # ---- constants / inputs to SBUF ----
            x_sb = cpool.tile([P, NFT, BC], dt.float16)
            for ft in range(NFT):
                nc.sync.dma_start(x_sb[:, ft, :], x16[:, ft, :])
            wpre_sb = cpool.tile([P, NFT, NW], dt.float16)
            nc.sync.dma_start(wpre_sb, wpre[:])
            bpre_sb = cpool.tile([NW, 1], dt.float32)
            nc.sync.dma_start(bpre_sb, bpre[:])
            bsin_sb = cpool.tile([NW, 1], dt.float32)
            nc.sync.dma_start(bsin_sb, bsin[:])
            eps_sb = cpool.tile([NW, 1], dt.float32)
            nc.sync.dma_start(eps_sb, eps[:])
            bits_sb = cpool.tile([NW, DIM], dt.float16)
            nc.sync.dma_start(bits_sb, bits[:])
            d_sb = cpool.tile([P, NKT, N_CLS], dt.float16)
            nc.sync.dma_start(d_sb, d16[:])
            bpost_sb = cpool.tile([N_CLS, 1], dt.float32)
            nc.sync.dma_start(bpost_sb, bpost[:])
            a_sb = cpool.tile([P, NKT, DIM], dt.float16)
            for kt in range(NKT):
                nc.sync.dma_start(a_sb[:, kt, :], a16[:, kt, :])

            q20 = cpool.tile([NW, BC], dt.float32)
            v20 = cpool.tile([NW, BC], dt.float32)
            lv20 = cpool.tile([NW, BC], dt.float16)
            s2T = cpool.tile([P, NKT, BC], dt.float16)
            p16 = cpool.tile([P, NKT, BC], dt.float16)
            outT_sb = cpool.tile([N_CLS, BC], dt.float32)

            # ---- prenet: pre[q, s] = Wpre @ X^T (q duplicated 2x), +bpre, tanh ----
            for ch in range(NCH):
                csl = slice(ch * CW, (ch + 1) * CW)
                pre_ps = ps_a.tile([NW, CW], dt.float32, name=f"pre{ch}", tag="ps")
                for ft in range(NFT):
                    nc.tensor.matmul(
                        pre_ps, wpre_sb[:, ft, :], x_sb[:, ft, csl],
                        start=(ft == 0), stop=(ft == NFT - 1),
                    )
                nc.scalar.activation(
                    q20[:, csl], pre_ps, AF.Tanh, bias=bpre_sb[:, 0:1]
                )

            # ---- rows 0-9: cos(phi) = sin(pi/4 q + 3pi/4); rows 10-19: sin(phi) ----
            nc.scalar.activation(
                v20, q20, AF.Sin, bias=bsin_sb[:, 0:1], scale=PI / 4.0
            )
            # lv = ln(v + 1e-5), fp16  (table switch #1: silu set -> ln/exp set)
            nc.scalar.activation(lv20, v20, AF.Ln, bias=eps_sb[:, 0:1])

            # ---- S2^T = exp(Bits @ lv), amplitude-major, fp16 ----
            for ch in range(NCH):
                csl = slice(ch * CW, (ch + 1) * CW)
                for kt in range(NKT):
                    ksl = slice(kt * P, (kt + 1) * P)
                    s2log = ps_a.tile([P, CW], dt.float32, name=f"s2l{ch}_{kt}", tag="ps")
                    nc.tensor.matmul(s2log, bits_sb[:, ksl], lv20[:, csl])
                    nc.scalar.activation(s2T[:, kt, csl], s2log, AF.Exp)

            # ---- main: Y^T = A @ S2^T (per jt), square on DVE ----
            for ch in range(NCH):
                csl = slice(ch * CW, (ch + 1) * CW)
                for jt in range(NKT):
                    jsl = slice(jt * P, (jt + 1) * P)
                    y_ps = ps_y.tile([P, CW], dt.float32, name=f"y{ch}_{jt}", tag="y")
                    for kt in range(NKT):
                        nc.tensor.matmul(
                            y_ps, a_sb[:, kt, jsl], s2T[:, kt, csl],
                            start=(kt == 0), stop=(kt == NKT - 1),
                        )
                    nc.scalar.activation(p16[:, jt, csl], y_ps, AF.Square)

            # ---- out^T = d @ (Y^T)^2 + bpost ----
            for ch in range(NCH):
                csl = slice(ch * CW, (ch + 1) * CW)
                out_ps = ps_o.tile([N_CLS, CW], dt.float32, name=f"o{ch}", tag="o")
                for jt in range(NKT):
                    nc.tensor.matmul(
                        out_ps, d_sb[:, jt, :], p16[:, jt, csl],
                        start=(jt == 0), stop=(jt == NKT - 1),
                    )
                nc.scalar.activation(
                    outT_sb[:, csl], out_ps, AF.Identity, bias=bpost_sb[:, 0:1]
                )

            nc.sync.dma_start(outT[:], outT_sb)

    nc.finalize()
    return nc


def _get_nc():
    if "nc" not in _CACHE:
        _CACHE["nc"] = _build_bass()
    return _CACHE["nc"]


def _pack_pmajor(arr, p=P):
    """(R, C) -> (p, R//p, C) partition-major: row r -> [r % ... ] tiles of p."""
    R, C = arr.shape
    return np.ascontiguousarray(
        arr.reshape(R // p, p, C).transpose(1, 0, 2)
    )


def _prepare_in_maps(input_features, W_pre, b_pre, q_params, W_post, b_post):
    X = np.asarray(input_features, np.float32)
    A = _build_A(q_params)
    aT16 = _pack_pmajor(np.ascontiguousarray(A.T).astype(np.float16))  # [128,8,1024]

    j = np.arange(DIM)
    bitmat = np.stack(
        [(j >> (N_QUBITS - 1 - w)) & 1 for w in range(N_QUBITS)]
    )  # (10, 1024)
    bits = np.concatenate([1.0 - bitmat, bitmat], axis=0).astype(np.float16)  # (20,1024)
    sgn = 1.0 - 2.0 * bitmat
    d = np.asarray(W_post, np.float64) @ sgn  # (2, 1024)
    dT16 = _pack_pmajor(np.ascontiguousarray(d.T).astype(np.float16))  # [128,8,2]

    WT = np.asarray(W_pre, np.float16).T  # (512, 10)
    wpre_pack = _pack_pmajor(
        np.ascontiguousarray(np.concatenate([WT, WT], axis=1))
    )  # [128,4,20]
    bpre_col = np.ascontiguousarray(
        np.tile(np.asarray(b_pre, np.float32), 2).reshape(NW, 1)
    )
    bsin_col = np.asarray(
        [3 * np.pi / 4] * N_QUBITS + [np.pi / 4] * N_QUBITS, np.float32
    ).reshape(NW, 1)
    eps_col = np.full((NW, 1), 1e-5, np.float32)
    bpost_col = np.asarray(b_post, np.float32).reshape(N_CLS, 1)

    XT16 = np.asarray(X, np.float16).T  # (512, 8192)
    in_maps = []
    for c in range(N_CORES):
        in_maps.append({
            "x16": _pack_pmajor(
                np.ascontiguousarray(XT16[:, c * BC:(c + 1) * BC])
            ),  # [128,4,1024]
            "wpre": wpre_pack,
            "bpre": bpre_col,
            "bsin": bsin_col,
            "eps": eps_col,
            "bits": bits,
            "a16": aT16,
            "d16": dT16,
            "bpost": bpost_col,
        })
    return in_maps


def run(inputs, trace=False):
    """Run on 8 cores; returns (output (8192, 2) f32, BassKernelResults)."""
    from concourse.bass_utils import run_bass_kernel_spmd

    nc = _get_nc()
    in_maps = _prepare_in_maps(**inputs)
    res = run_bass_kernel_spmd(
        nc, in_maps, core_ids=list(range(N_CORES)), trace=trace
    )
    out = np.empty((B_FULL, N_CLS), np.float32)
    for c in range(N_CORES):
        out[c * BC:(c + 1) * BC, :] = res.results[c]["outT"].T
    return out, res


def kernel(input_features, W_pre, b_pre, q_params, W_post, b_post):
    out, _ = run(dict(
        input_features=input_features, W_pre=W_pre, b_pre=b_pre,
        q_params=q_params, W_post=W_post, b_post=b_post,
    ))
    return out


# revision 15
# speedup vs baseline: 1.9980x; 1.0883x over previous
"""Trainium2 Bass kernel for a hybrid classical/quantum head.

Math: the reference is  out = Q(tanh(X @ Wpre.T + bpre) * pi/2) @ Wpost.T + bpost
where Q() simulates a 10-qubit circuit: H on all wires, per-sample RY(theta_w),
then 6 layers of (CNOT chain + shared RY(qw)), returning PauliZ expvals.

Restructuring used here:
  * After H + per-sample RY, the state is a PRODUCT state:
      s2[j] = prod_w v_w(bit_w(j)),  v_w(0)=cos(phi_w), v_w(1)=sin(phi_w),
      phi_w = theta_w/2 + pi/4,  theta_w = tanh(pre)*pi/2.
    All v are strictly positive (phi in (0, pi/2)), so the product state can
    be built in AMPLITUDE-MAJOR layout directly via a log-domain matmul:
      S2^T[k, s] = exp( sum_r Bits[k, r] * ln(v_r[s]) )
    with Bits a constant (1024, 20) 0/1 bit-selection matrix.  This kills
    the 64 serialized DMA transposes that dominated the previous version.
  * Everything after the per-sample RY layer is a fixed linear operator A
    (1024x1024) depending only on q_params -> built host-side in fp64,
    shipped as fp16.
  * z_w = sum_j sign_w(j) * (A s2)_j^2, and the post-linear folds in:
      out[s, c] = sum_j d[c, j] * y[s, j]^2 + bpost[c],  d = Wpost @ Sgn.

Per-core device pipeline (data-parallel over batch, 1024 samples/core):
  prenet matmul (Wpre stationary, fp16 PE) -> [10,1024] tanh/sin/ln ACT chain
  -> Bits matmul (PE) -> exp (ACT) -> S2^T fp16
  -> Y^T = A @ S2^T (fp16 PE matmul, f32 accum) -> square (DVE)
  -> d-contraction (fp16 PE matmul) -> +bias (ACT Identity) -> out^T f32.
"""

import numpy as np

N_QUBITS = 10
Q_DEPTH = 6
MAX_LAYERS = 15
DIM = 2**N_QUBITS
N_CORES = 8
B_FULL = 8192
F_IN = 512
N_CLS = 2
BC = B_FULL // N_CORES  # 1024 samples per core
P = 128

_CACHE = {}


def _build_A(q_params):
    """Fixed circuit operator after the per-sample RY layer, fp64 on host."""
    qp = np.asarray(q_params, np.float64)
    qw = qp.reshape(MAX_LAYERS, N_QUBITS)
    N = N_QUBITS

    def apply_1q(M, U, w):
        a, b = 2**w, 2 ** (N - 1 - w)
        M = M.reshape(a, 2, b, DIM)
        M = np.einsum('ij,ajbk->aibk', U, M)
        return M.reshape(DIM, DIM)

    def apply_cnot(M, c, t):
        M = M.reshape(2**c, 2, 2 ** (t - c - 1), 2, 2 ** (N - 1 - t), DIM)
        M = np.stack([M[:, 0], np.flip(M[:, 1], axis=2)], axis=1)
        return M.reshape(DIM, DIM)

    def ry(th):
        c, s = np.cos(th / 2), np.sin(th / 2)
        return np.array([[c, -s], [s, c]])

    A = np.eye(DIM)
    for k in range(Q_DEPTH):
        for i in range(0, N - 1, 2):
            A = apply_cnot(A, i, i + 1)
        for i in range(1, N - 1, 2):
            A = apply_cnot(A, i, i + 1)
        for w in range(N):
            A = apply_1q(A, ry(qw[k + 1, w]), w)
    return A


NKT = DIM // P  # 8 amplitude tiles
NFT = F_IN // P  # 4 feature tiles
NCH = 2  # two 512-wide sample chunks (PSUM bank = 512 f32)
CW = BC // NCH  # 512
NW = 2 * N_QUBITS  # 20 selection rows


def _build_bass():
    import concourse.mybir as mybir
    from concourse import bacc
    from concourse.tile import TileContext

    dt = mybir.dt
    AF = mybir.ActivationFunctionType
    PI = float(np.pi)

    nc = bacc.Bacc()
    # blob16 packs (per-partition, fp16): wpre [*, 4*20], bits [*, 8*128]
    # (junk below row 20), d [*, 8*2].  blob32 packs the [20, 1] f32 bias
    # columns: bpre, bsin, eps, bpost (rows 0-1).  One dma_start each --
    # DIRECT2D dispatches serialize at ~650ns on the Sync sequencer, so
    # fewer, larger DMAs shorten the head.
    W_OFF, B_OFF, D_OFF = 0, NFT * NW, NFT * NW + DIM
    BLOB16_W = D_OFF + NKT * N_CLS
    blob16 = nc.dram_tensor("blob16", [P, BLOB16_W], dt.float16, kind="ExternalInput")
    blob32 = nc.dram_tensor("blob32", [NW, 4], dt.float32, kind="ExternalInput")
    xin = nc.dram_tensor("xin", [NCH, P, NFT, CW], dt.float16, kind="ExternalInput")
    a16 = nc.dram_tensor("a16", [P, NKT, DIM], dt.float16, kind="ExternalInput")
    outT = nc.dram_tensor("outT", [N_CLS, BC], dt.float32, kind="ExternalOutput")

    with TileContext(nc) as tc:
        with (
            tc.tile_pool(name="const", bufs=1) as cpool,
            tc.tile_pool(name="ps_a", bufs=2, space="PSUM") as ps_a,
            tc.tile_pool(name="ps_y", bufs=4, space="PSUM") as ps_y,
            tc.tile_pool(name="ps_o", bufs=2, space="PSUM") as ps_o,
        ):
            # ---- inputs to SBUF: blob16, x halves, blob32, then A ----
            blob16_sb = cpool.tile([P, BLOB16_W], dt.float16)
            nc.sync.dma_start(blob16_sb, blob16[:])
            x_sb = cpool.tile([P, NFT, BC], dt.float16)
            for ch in range(NCH):
                nc.sync.dma_start(
                    x_sb[:, :, ch * CW:(ch + 1) * CW], xin[ch]
                )
            blob32_sb = cpool.tile([NW, 4], dt.float32)
            nc.sync.dma_start(blob32_sb, blob32[:])
            a_sb = cpool.tile([P, NKT, DIM], dt.float16)
            nc.sync.dma_start(a_sb, a16[:])

            def wpre_ap(ft):
                return blob16_sb[:, W_OFF + ft * NW:W_OFF + (ft + 1) * NW]

            def bits_ap(kt):
                return blob16_sb[0:NW, B_OFF + kt * P:B_OFF + (kt + 1) * P]

            def d_ap(jt):
                return blob16_sb[:, D_OFF + jt * N_CLS:D_OFF + (jt + 1) * N_CLS]

            bpre_b = blob32_sb[:, 0:1]
            bsin_b = blob32_sb[:, 1:2]
            eps_b = blob32_sb[:, 2:3]
            bpost_b = blob32_sb[0:N_CLS, 3:4]

            # ---- PE warm-up: dummy matmuls ramp the p-state while DMAs run ----
            warm_in = cpool.tile([P, P], dt.float16)
            nc.gpsimd.memset(warm_in, 0.0)
            warm_ps = ps_y.tile([P, P], dt.float32, name="warm", tag="y")
            for _ in range(16):
                nc.tensor.matmul(warm_ps, warm_in, warm_in)

            q20 = cpool.tile([NW, BC], dt.float32)
            v20 = cpool.tile([NW, BC], dt.float32)
            lv20 = cpool.tile([NW, BC], dt.float16)
            s2T = cpool.tile([P, NKT, BC], dt.float16)
            p16 = cpool.tile([P, NKT, BC], dt.float16)
            outT_sb = cpool.tile([N_CLS, BC], dt.float32)

            # ---- prenet: pre[q, s] = Wpre @ X^T (q duplicated 2x), +bpre, tanh ----
            for ch in range(NCH):
                csl = slice(ch * CW, (ch + 1) * CW)
                pre_ps = ps_a.tile([NW, CW], dt.float32, name=f"pre{ch}", tag="ps")
                for ft in range(NFT):
                    nc.tensor.matmul(
                        pre_ps, wpre_ap(ft), x_sb[:, ft, csl],
                        start=(ft == 0), stop=(ft == NFT - 1),
                    )
                nc.scalar.activation(
                    q20[:, csl], pre_ps, AF.Tanh, bias=bpre_b
                )

            # ---- rows 0-9: cos(phi) = sin(pi/4 q + 3pi/4); rows 10-19: sin(phi) ----
            nc.scalar.activation(
                v20, q20, AF.Sin, bias=bsin_b, scale=PI / 4.0
            )
            # lv = ln(v + 1e-5), fp16  (table switch #1: silu set -> ln/exp set)
            nc.scalar.activation(lv20, v20, AF.Ln, bias=eps_b)

            # ---- S2^T = exp(Bits @ lv), amplitude-major, fp16 ----
            for ch in range(NCH):
                csl = slice(ch * CW, (ch + 1) * CW)
                for kt in range(NKT):
                    ksl = slice(kt * P, (kt + 1) * P)
                    s2log = ps_a.tile([P, CW], dt.float32, name=f"s2l{ch}_{kt}", tag="ps")
                    nc.tensor.matmul(s2log, bits_ap(kt), lv20[:, csl])
                    nc.scalar.activation(s2T[:, kt, csl], s2log, AF.Exp)

            # ---- main: Y^T = A @ S2^T (per jt), square on DVE ----
            for ch in range(NCH):
                csl = slice(ch * CW, (ch + 1) * CW)
                for jt in range(NKT):
                    jsl = slice(jt * P, (jt + 1) * P)
                    y_ps = ps_y.tile([P, CW], dt.float32, name=f"y{ch}_{jt}", tag="y")
                    for kt in range(NKT):
                        nc.tensor.matmul(
                            y_ps, a_sb[:, kt, jsl], s2T[:, kt, csl],
                            start=(kt == 0), stop=(kt == NKT - 1),
                        )
                    nc.scalar.activation(p16[:, jt, csl], y_ps, AF.Square)

            # ---- out^T = d @ (Y^T)^2 + bpost ----
            for ch in range(NCH):
                csl = slice(ch * CW, (ch + 1) * CW)
                out_ps = ps_o.tile([N_CLS, CW], dt.float32, name=f"o{ch}", tag="o")
                for jt in range(NKT):
                    nc.tensor.matmul(
                        out_ps, d_ap(jt), p16[:, jt, csl],
                        start=(jt == 0), stop=(jt == NKT - 1),
                    )
                nc.scalar.activation(
                    outT_sb[:, csl], out_ps, AF.Identity, bias=bpost_b
                )

            nc.sync.dma_start(outT[:], outT_sb)

    nc.finalize()
    return nc


def _get_nc():
    if "nc" not in _CACHE:
        _CACHE["nc"] = _build_bass()
    return _CACHE["nc"]


def _pack_pmajor(arr, p=P):
    """(R, C) -> (p, R//p, C) partition-major: row r -> [r % ... ] tiles of p."""
    R, C = arr.shape
    return np.ascontiguousarray(
        arr.reshape(R // p, p, C).transpose(1, 0, 2)
    )


def _prepare_in_maps(input_features, W_pre, b_pre, q_params, W_post, b_post):
    X = np.asarray(input_features, np.float32)
    A = _build_A(q_params)
    aT16 = _pack_pmajor(np.ascontiguousarray(A.T).astype(np.float16))  # [128,8,1024]

    j = np.arange(DIM)
    bitmat = np.stack(
        [(j >> (N_QUBITS - 1 - w)) & 1 for w in range(N_QUBITS)]
    )  # (10, 1024)
    bits = np.concatenate([1.0 - bitmat, bitmat], axis=0).astype(np.float16)  # (20,1024)
    sgn = 1.0 - 2.0 * bitmat
    d = np.asarray(W_post, np.float64) @ sgn  # (2, 1024)
    dT16 = _pack_pmajor(np.ascontiguousarray(d.T).astype(np.float16))  # [128,8,2]

    WT = np.asarray(W_pre, np.float16).T  # (512, 10)
    wpre_pack = _pack_pmajor(
        np.ascontiguousarray(np.concatenate([WT, WT], axis=1))
    )  # [128,4,20]

    # blob16: [128, 4*20 | 8*128 (bits, junk rows >= 20) | 8*2 (d)]
    bits_pad = np.zeros((P, DIM), np.float16)
    bits_pad[0:NW, :] = bits
    blob16 = np.concatenate([
        wpre_pack.reshape(P, NFT * NW),
        bits_pad,
        dT16.reshape(P, NKT * N_CLS),
    ], axis=1)
    blob16 = np.ascontiguousarray(blob16)

    # blob32 columns: bpre (2x), bsin, eps, bpost (rows 0-1)
    blob32 = np.zeros((NW, 4), np.float32)
    blob32[:, 0] = np.tile(np.asarray(b_pre, np.float32), 2)
    blob32[0:N_QUBITS, 1] = 3 * np.pi / 4
    blob32[N_QUBITS:NW, 1] = np.pi / 4
    blob32[:, 2] = 1e-5
    blob32[0:N_CLS, 3] = np.asarray(b_post, np.float32)

    XT16 = np.asarray(X, np.float16).T  # (512, 8192)
    in_maps = []
    for c in range(N_CORES):
        xc = _pack_pmajor(
            np.ascontiguousarray(XT16[:, c * BC:(c + 1) * BC])
        )  # [128,4,1024]
        xin = np.ascontiguousarray(
            np.stack([xc[:, :, 0:CW], xc[:, :, CW:BC]], axis=0)
        )  # [2,128,4,512]
        in_maps.append({
            "xin": xin,
            "blob16": blob16,
            "blob32": blob32,
            "a16": aT16,
        })
    return in_maps


def run(inputs, trace=False):
    """Run on 8 cores; returns (output (8192, 2) f32, BassKernelResults)."""
    from concourse.bass_utils import run_bass_kernel_spmd

    nc = _get_nc()
    in_maps = _prepare_in_maps(**inputs)
    res = run_bass_kernel_spmd(
        nc, in_maps, core_ids=list(range(N_CORES)), trace=trace
    )
    out = np.empty((B_FULL, N_CLS), np.float32)
    for c in range(N_CORES):
        out[c * BC:(c + 1) * BC, :] = res.results[c]["outT"].T
    return out, res


def kernel(input_features, W_pre, b_pre, q_params, W_post, b_post):
    out, _ = run(dict(
        input_features=input_features, W_pre=W_pre, b_pre=b_pre,
        q_params=q_params, W_post=W_post, b_post=b_post,
    ))
    return out
